# revision 9
# baseline (speedup 1.0000x reference)
"""Trainium2 Bass kernel for NeuralTensorLayer (order-1/2/3 polynomial layer).

    out[b,l] = bias[l] + sum_i X[b,i] W1[i,l]
             + sum_ij X[b,i] X[b,j] W2[i,j,l]
             + sum_ijk X[b,i] X[b,j] X[b,k] W3[i,j,k,l]

with B=32768, D=K=32, data-parallel over 8 NeuronCores (4096 rows each).

Strategy (per core):
  * Exploit (i,j) symmetry: only the 528 pairs i<=j are needed against
    host-symmetrized weights W3s[ij,k,l] = W3[i,j,k,l]+W3[j,i,k,l] (i<j),
    cutting the dominant matmul contraction from 1024 -> 528 (+32 X rows).
  * Pair operands arrive host-gathered in transposed layout and batched per
    supertile: XEB/XRB [s, 128, 4*512] hold chunks 0-3 side by side (one
    big DMA each; supertile 0 is split per-chunk so the PE starts early).
    One DVE multiply builds Z^T for chunks 0-3 at 2x.  Chunk 4 (16 pairs +
    32 order-1 X rows) is built twice: at partitions 0:48 for even tiles
    and 64:112 for odd tiles, so each tile pair's chunk-4 matmuls run
    CONCURRENTLY in different PE row groups (h0/h1 via tile_position).
  * Per 128-row tile, one fp32-PSUM matmul group over two regions:
    A[b, l*32+k] (order-3 grid, 1024 cols = 2 psum banks, 3 bufs) and
    OB[b, t*32+l] (order-1+2 "out_low", 32 cols in a shared per-supertile
    psum tile).  Chunk-4 matmuls of a tile pair are emitted interleaved
    after both tiles' chunk 0-3 groups.
  * Post, spread across engines: ACT stages A to SBUF bf16; DVE multiplies
    by the X broadcast (2x); GPSIMD folds k 32->16 (aligned add); DVE
    reduces 16->1 and adds the out_low psum slice; bias added on host.
  * Input DMAs are split across the SYNC and GPSIMD queues (the sync
    sequencer alone bottlenecks at ~0.7us per DMA issue).
"""

import numpy as np
import ml_dtypes
from contextlib import ExitStack

import concourse.bass as bass
import concourse.bacc as bacc
import concourse.tile as tile
from concourse import mybir
from concourse import bass_utils

BF16 = ml_dtypes.bfloat16

B, D, KOUT = 32768, 32, 32
NCORES = 8
BLOC = B // NCORES          # 4096 rows per core
P = 128                     # rows per tile
SUPER = 4                   # tiles per supertile
SP = SUPER * P              # 512
SP2 = SP // 2               # 256
NSUPER = BLOC // SP         # 8
NPAIRS = D * (D + 1) // 2   # 528
NCA = KOUT * D              # 1024 order-3 psum columns (A region)
NCOL = NCA + KOUT           # 1056 weight columns (A grid + out_low)
KF = 16                     # folded k width (32 -> 16 via one aligned add)

PAIRS = [(i, j) for i in range(D) for j in range(i, D)]
I_P = np.array([p[0] for p in PAIRS], np.int32)
J_P = np.array([p[1] for p in PAIRS], np.int32)

# chunk-4 column permutation (identity: pairing disabled)
TPERM = [0, 1, 2, 3]

F32 = mybir.dt.float32
BF = mybir.dt.bfloat16


# Drop redundant LDWEIGHTS from the BIR before walrus codegen: matmuls that
# share a stationary operand (the three N-splits per contraction chunk)
# each carry their own Ldweights (walrus's ldw-opt pass is disabled/broken).
# A load is elided when the previous PE weight-op in SCHEDULED order has a
# byte-identical weight AP and the load itself carries no semaphore
# waits/updates (so the PE weight registers provably still hold the same
# data and no sync edge is lost).
def _dedup_ldweights(bir_json: bytes) -> bytes:
    import json as _json

    d = _json.loads(bir_json)
    for fn in d.get("functions", []):
        for blk in fn.get("blocks", []):
            out = []
            last = None
            for i in blk.get("instructions", []):
                if i.get("engine") == "PE" and i.get("opcode") in ("Ldweights", "Matmult"):
                    w = i["ins"][-1] if i["opcode"] == "Matmult" else i["ins"][0]
                    key = (w.get("memref"), w.get("offset"), _json.dumps(w.get("ap")),
                           w.get("dtype"), _json.dumps(i.get("tile_position")),
                           _json.dumps(i.get("tile_size")), i.get("perf_mode"))
                    if i["opcode"] == "Ldweights":
                        si = i.get("sync_info") or {}
                        if (key == last and not si.get("on_wait")
                                and not si.get("on_update")):
                            continue
                        last = key
                    else:
                        # a Matmult's weight ref mirrors the loaded state
                        # (self-loading or not), so it may refresh `last`
                        last = key
                elif i.get("engine") == "PE":
                    last = None  # unknown PE op: invalidate weight-reuse state
                out.append(i)
            blk["instructions"] = out
    return _json.dumps(d).encode()


if not getattr(bass_utils, "_ldw_dedup_patched", False):
    _orig_compile_bir_kernel = bass_utils.compile_bir_kernel

    def _compile_bir_kernel_dedup(bir_json, tmpdir, neff_name="file.neff"):
        return _orig_compile_bir_kernel(_dedup_ldweights(bir_json), tmpdir, neff_name)

    bass_utils.compile_bir_kernel = _compile_bir_kernel_dedup
    import concourse.bass2jax as _b2j

    _b2j.compile_bir_kernel = _compile_bir_kernel_dedup
    bass_utils._ldw_dedup_patched = True


def _pack_weights(W1, W2, W3):
    W1 = np.asarray(W1, np.float64)
    W2 = np.asarray(W2, np.float64)
    W3 = np.asarray(W3, np.float64)
    Wcat = np.zeros((5, 128, NCOL), np.float64)
    for p, (i, j) in enumerate(PAIRS):
        c, pp = divmod(p, 128)
        if i < j:
            w3 = W3[i, j] + W3[j, i]   # [k, l]
            w2 = W2[i, j] + W2[j, i]   # [l]
        else:
            w3 = W3[i, i]
            w2 = W2[i, i]
        Wcat[c, pp, :NCA] = w3.T.reshape(-1)   # col l*32+k
        Wcat[c, pp, NCA:] = w2                 # out_low columns
    for dd in range(D):                # order-1: X rows in chunk 4
        Wcat[4, 16 + dd, NCA:] = W1[dd]
    return Wcat.astype(np.float32).astype(BF16)


def _build_module():
    nc = bacc.Bacc("TRN2", target_bir_lowering=False, debug=False,
                   enable_asserts=False)
    XBd = nc.dram_tensor("XB", [BLOC, D], BF, kind="ExternalInput").ap()
    # chunk-4 operands, tile columns permuted [t0, t2, t1, t3]
    XT4d = nc.dram_tensor("XT4", [NSUPER, D, SP], BF, kind="ExternalInput").ap()
    XE4d = nc.dram_tensor("XE4", [NSUPER, 16, SP], BF, kind="ExternalInput").ap()
    XR4d = nc.dram_tensor("XR4", [NSUPER, 16, SP], BF, kind="ExternalInput").ap()
    # chunks 0-3 batched per supertile
    XEBd = nc.dram_tensor("XEB", [NSUPER, 128, 4 * SP], BF, kind="ExternalInput").ap()
    XRBd = nc.dram_tensor("XRB", [NSUPER, 128, 4 * SP], BF, kind="ExternalInput").ap()
    WCd = nc.dram_tensor("WCAT", [5, 128, NCOL], BF, kind="ExternalInput").ap()
    OUTd = nc.dram_tensor("OUT", [BLOC, KOUT], F32, kind="ExternalOutput").ap()

    with ExitStack() as ctx:
        tc = ctx.enter_context(tile.TileContext(nc))
        consts = ctx.enter_context(tc.tile_pool(name="consts", bufs=1))
        xbpool = ctx.enter_context(tc.tile_pool(name="xbpool", bufs=3 * SUPER))
        xepool = ctx.enter_context(tc.tile_pool(name="xepool", bufs=3))
        zpool = ctx.enter_context(tc.tile_pool(name="zpool", bufs=3))
        spool = ctx.enter_context(tc.tile_pool(name="spool", bufs=4))
        upool = ctx.enter_context(tc.tile_pool(name="upool", bufs=3))
        fpool = ctx.enter_context(tc.tile_pool(name="fpool", bufs=3))
        tpool = ctx.enter_context(tc.tile_pool(name="tpool", bufs=4))
        opool = ctx.enter_context(tc.tile_pool(name="opool", bufs=4))
        t3ps = ctx.enter_context(tc.tile_pool(name="t3ps", bufs=3, space="PSUM"))
        obps = ctx.enter_context(tc.tile_pool(name="obps", bufs=2, space="PSUM"))

        w_sb = []
        for c in range(5):
            w = consts.tile([128, NCOL], BF, tag=f"w_{c}")
            nc.scalar.dma_start(out=w, in_=WCd[c])
            w_sb.append(w)


        def build(s):
            """DMA x tiles and build Z^T chunks for supertile s."""
            row0 = s * SP
            xeb = xepool.tile([128, 4 * SP], BF, tag="xeb")
            xrb = xepool.tile([128, 4 * SP], BF, tag="xrb")
            if s == 0:
                # per-chunk DMAs so the first matmul can start early
                for c in range(4):
                    nc.sync.dma_start(out=xeb[:, c * SP:(c + 1) * SP],
                                      in_=XEBd[0, :, c * SP:(c + 1) * SP])
                    nc.gpsimd.dma_start(out=xrb[:, c * SP:(c + 1) * SP],
                                        in_=XRBd[0, :, c * SP:(c + 1) * SP])
            else:
                nc.sync.dma_start(out=xeb, in_=XEBd[s])
                nc.gpsimd.dma_start(out=xrb, in_=XRBd[s])
            xe4 = xepool.tile([16, SP], BF, tag="xe4")
            nc.sync.dma_start(out=xe4, in_=XE4d[s])
            xr4 = xepool.tile([16, SP], BF, tag="xr4")
            nc.gpsimd.dma_start(out=xr4, in_=XR4d[s])
            xbs = []
            for t in range(SUPER):
                xb = xbpool.tile([P, D], BF, tag="xb")
                nc.sync.dma_start(out=xb, in_=XBd[row0 + t * P: row0 + (t + 1) * P, :])
                xbs.append(xb)
            zb = zpool.tile([128, 4 * SP], BF, tag="zb")
            if s == 0:
                for c in range(4):
                    nc.vector.tensor_mul(zb[:, c * SP:(c + 1) * SP],
                                         xeb[:, c * SP:(c + 1) * SP],
                                         xrb[:, c * SP:(c + 1) * SP])
            else:
                nc.vector.tensor_mul(zb, xeb, xrb)
            # chunk 4: 16 pairs + 32 order-1 X rows
            z4 = zpool.tile([48, SP], BF, tag="z4")
            nc.vector.tensor_mul(z4[:16], xe4, xr4)
            nc.gpsimd.dma_start(out=z4[16:48, :], in_=XT4d[s])
            return xbs, zb, z4

        NSPLITS_A = ((0, 512), (512, 1024))

        def post(t3, ob, xb, tslot, row0, t, direct=False):
            """Stage/multiply/fold/reduce one tile's psum into OUT."""
            if direct:
                src = t3            # read psum directly (tail latency path)
                sk = D
            else:
                staged = spool.tile([P, NCA], BF, tag="staged")
                nc.scalar.copy(out=staged, in_=t3)
                src = staged
                sk = D
            u = upool.tile([P, NCA], BF, tag="u")
            xk = xb[:, :].unsqueeze(1).broadcast_to([P, KOUT, D])
            nc.vector.tensor_mul(
                u[:, :].rearrange("p (l k) -> p l k", k=sk),
                src[:, :].rearrange("p (l k) -> p l k", k=sk),
                xk,
            )
            # fold k 32 -> 16 (aligned, on GPSIMD), then DVE reduce + out_low
            f = fpool.tile([P, KOUT * KF], BF, tag="f")
            ur = u[:, :].rearrange("p (l k) -> p l k", k=D)
            eng = nc.vector if direct else nc.gpsimd
            eng.tensor_add(
                f[:, :].rearrange("p (l k) -> p l k", k=KF),
                ur[:, :, 0:KF],
                ur[:, :, KF:D],
            )
            red = tpool.tile([P, KOUT], F32, tag="red")
            nc.vector.reduce_sum(
                out=red, in_=f[:, :].rearrange("p (l k) -> p l k", k=KF),
                axis=mybir.AxisListType.X,
            )
            osb = opool.tile([P, KOUT], F32, tag="osb")
            nc.vector.tensor_add(osb, red, ob[:, tslot * KOUT:(tslot + 1) * KOUT])
            nc.scalar.dma_start(out=OUTd[row0 + t * P: row0 + (t + 1) * P, :],
                                in_=osb)

        state = build(0)
        for s in range(NSUPER):
            xbs, zb, z4 = state
            if s + 1 < NSUPER:
                state = build(s + 1)
            row0 = s * SP
            ob = obps.tile([P, SUPER * KOUT], F32, tag="ob")
            for t in range(SUPER):
                t3 = t3ps.tile([P, NCA], F32, tag="t3")
                for c in range(5):
                    if c < 4:
                        zc = zb[:, c * SP + t * P: c * SP + (t + 1) * P]
                        wc = w_sb[c]
                    else:
                        zc = z4[:, t * P: (t + 1) * P]
                        wc = w_sb[4][:48]
                    for n0, n1 in NSPLITS_A:
                        nc.tensor.matmul(t3[:, n0:n1], zc, wc[:, n0:n1],
                                         start=(c == 0), stop=(c == 4))
                    nc.tensor.matmul(ob[:, t * KOUT:(t + 1) * KOUT], zc,
                                     wc[:, NCA:],
                                     start=(c == 0), stop=(c == 4))
                last_tile = (s == NSUPER - 1 and t == SUPER - 1)
                post(t3, ob, xbs[t], t, row0, t, direct=last_tile)
    nc.compile()
    return nc


_CACHE = {}


def _get_module():
    if "nc" not in _CACHE:
        _CACHE["nc"] = _build_module()
    return _CACHE["nc"]


def kernel(X, W1, W2, W3, bias):
    X = np.ascontiguousarray(np.asarray(X, np.float32))
    bias = np.asarray(bias, np.float32)
    Wcat = _pack_weights(W1, W2, W3)

    nc = _get_module()
    Xb = X.astype(BF16)                      # [B, D] bf16 (single rounding point)
    XbT = np.ascontiguousarray(Xb.T)         # [D, B] bf16
    XE = XbT[I_P]                            # [528, B]
    XR = XbT[J_P]
    # chunks 0-3 batched per supertile: [ncores, NSUPER, 128, 4*SP]
    def _batch(A):
        Ab = A[:512].reshape(4, 128, NCORES, NSUPER, SP)
        big = np.ascontiguousarray(Ab.transpose(2, 3, 1, 0, 4).reshape(NCORES, NSUPER, 128, 4 * SP))
        A4 = A[512:528].reshape(16, NCORES, NSUPER, SUPER, P)
        c4 = np.ascontiguousarray(A4[:, :, :, TPERM, :].transpose(1, 2, 0, 3, 4).reshape(NCORES, NSUPER, 16, SP))
        return big, c4
    XEB, XE4 = _batch(XE)
    XRB, XR4 = _batch(XR)
    XT4 = np.ascontiguousarray(
        XbT.reshape(D, NCORES, NSUPER, SUPER, P)[:, :, :, TPERM, :]
        .transpose(1, 2, 0, 3, 4).reshape(NCORES, NSUPER, D, SP))
    shards = Xb.reshape(NCORES, BLOC, D)
    in_maps = [
        {
            "XB": np.ascontiguousarray(shards[c]),
            "XT4": XT4[c],
            "XEB": XEB[c], "XRB": XRB[c], "XE4": XE4[c], "XR4": XR4[c],
            "WCAT": Wcat,
        }
        for c in range(NCORES)
    ]
    res = bass_utils.run_bass_kernel_spmd(nc, in_maps, core_ids=list(range(NCORES)))
    _CACHE["last_results"] = res
    out = np.concatenate([np.asarray(res.results[c]["OUT"]) for c in range(NCORES)], 0)
    return (out + bias.reshape(1, KOUT)).astype(np.float32)
